# revision 1
# baseline (speedup 1.0000x reference)
"""Distributed multi-head attention kernel for 8 TRN2 NeuronCores.

Problem: x [4, 2048, 1024] -> qkv proj -> 16-head attention (d=64)
         -> out proj + bias -> [4, 2048, 1024].

Sharding (no collectives): core i handles batch b = i//2 and query-half
half = i%2 (1024 query tokens). Each core computes K/V for its batch's
full 2048-token sequence (duplicated within the pair of cores sharing a
batch) and Q only for its own 1024 tokens. The host rotates the token
axis per core so the core's query tokens are always tokens [0, 1024) of
its input -- attention is permutation-invariant over keys, so K/V token
order does not matter.

Per-core pipeline (everything bf16 on the TensorE, fp32 PSUM accum):
  proj:  Q^T [d, q] / K^T [d, k] head-pairs packed on 128 partitions;
         V [k, d] in 65-wide per-head blocks with a ones column
         (the PV matmul then yields softmax denominators for free).
  attn:  per head: S^T = K @ Q^T -> exp on ScalarE (x0.125 fused, no
         max subtraction; scores are O(1) by construction) -> bf16 P^T
         -> PV accumulation U^T[65, q]; row 64 = denominator.
         Tail: U^T -> SBUF bf16 + 1/D (fp16) immediately (frees PSUM);
         normalize = K=1 ones matmul broadcast + DVE multiply, off the
         critical path.
  out:   two passes (heads 0-7 + bias, then heads 8-15) accumulating
         through a DRAM scratch so pass A fills the PE during the
         ACT-bound attention of the second half.

The two halves' projections and attention phases are arranged so the
PE always has matmul work while the ScalarE grinds through exp()
(keeps the PE HAM clock gate at 2.4 GHz).
"""

import numpy as np
import ml_dtypes

B = 4
N = 2048
DIM = 1024
HEADS = 16
DH = 64
NQ = 1024  # query tokens per core
NCORES = 8

_CACHE = {}


def _build_nc():
    from contextlib import ExitStack

    import concourse.bass as bass
    import concourse.mybir as mybir
    import concourse.tile as tile
    from concourse import bacc

    f32 = mybir.dt.float32
    bf16 = mybir.dt.bfloat16
    f16 = mybir.dt.float16
    EXP = mybir.ActivationFunctionType.Exp

    nc = bacc.Bacc("TRN2", target_bir_lowering=False, debug=False,
                   num_devices=NCORES)

    xt_d = nc.dram_tensor("xt", [DIM, N], bf16, kind="ExternalInput")
    wqkv_d = nc.dram_tensor("wqkv", [DIM, 3 * DIM], bf16, kind="ExternalInput")
    wo_d = nc.dram_tensor("wo", [HEADS, DH, DIM], bf16, kind="ExternalInput")
    brow_d = nc.dram_tensor("brow", [1, DIM], bf16, kind="ExternalInput")
    out_d = nc.dram_tensor("out", [NQ, DIM], f32, kind="ExternalOutput")

    with tile.TileContext(nc) as tc, ExitStack() as top:
        const_pool = top.enter_context(tc.tile_pool(name="const", bufs=1))
        mm_psum = top.enter_context(tc.tile_pool(name="mmps", bufs=2, space="PSUM"))
        sp_psum = top.enter_context(tc.tile_pool(name="spps", bufs=2, space="PSUM"))
        u_psum = top.enter_context(tc.tile_pool(name="ups", bufs=1, space="PSUM"))
        es_pool = top.enter_context(tc.tile_pool(name="es", bufs=6))
        rec_pool = top.enter_context(tc.tile_pool(name="rec", bufs=4))
        bc_pool = top.enter_context(tc.tile_pool(name="bc", bufs=3))
        uraw_a = top.enter_context(tc.tile_pool(name="uraw_a", bufs=1))
        dram_pool = top.enter_context(tc.tile_pool(name="dscr", bufs=1, space="DRAM"))

        brow_t = const_pool.tile([1, DIM], bf16, tag="brow", name="brow")
        nc.sync.dma_start(brow_t[:], brow_d.ap()[:])
        ones_t = const_pool.tile([1, 128], bf16, tag="ones", name="ones")
        nc.gpsimd.memset(ones_t[:], 1.0)
        ones_bc = const_pool.tile([128, 64], f16, tag="ones_bc", name="ones_bc")
        nc.gpsimd.memset(ones_bc[:], 1.0)

        uraw = [None] * HEADS

        def proj_units(half, w_pool, xt, QT, KT, VT):
            """Emission closures, one PSUM-group each.

            Order: all of V, then K/Q alternating per head-pair chunk so
            early head pairs become ready as soon as possible.
            """
            def dma_factory(col0):
                box = [None]
                def dma():
                    if box[0] is None:
                        wb = [w_pool.tile([128, 512], bf16, tag=f"w{fc}",
                                          name=f"w{fc}") for fc in range(8)]
                        for fc in range(8):
                            nc.sync.dma_start(
                                wb[fc][:],
                                wqkv_d.ap()[fc * 128:(fc + 1) * 128,
                                            col0:col0 + 512])
                        box[0] = wb
                    return box[0]
                return dma

            dma_v = dma_factory(2 * DIM + half * 512)
            dma_k = dma_factory(DIM + half * 512)
            dma_q = dma_factory(half * 512)

            def v_unit(mk, dma=dma_v):
                wb = dma()
                ps = mm_psum.tile([128, 512], f32, tag="mm", name="mm")
                for fc in range(8):
                    nc.tensor.matmul(
                        ps[:], xt[fc][:, mk * 128:(mk + 1) * 128], wb[fc][:],
                        start=(fc == 0), stop=(fc == 7))
                nc.vector.tensor_copy(
                    VT[mk][:, :, 0:64],
                    ps[:].rearrange("p (h d) -> p h d", d=64))
                nc.gpsimd.memset(VT[mk][:, :, 64:65], 1.0)

            def qk_unit(dma, dest, m4, t):
                wb = dma()
                ps = mm_psum.tile([128, 512], f32, tag="mm", name="mm")
                for fc in range(8):
                    nc.tensor.matmul(
                        ps[:], wb[fc][:, m4 * 128:(m4 + 1) * 128],
                        xt[fc][:, t * 512:(t + 1) * 512],
                        start=(fc == 0), stop=(fc == 7))
                nc.vector.tensor_copy(
                    dest[m4][:, t * 512:(t + 1) * 512], ps[:])

            units = [lambda mk=mk: v_unit(mk) for mk in range(16)]
            for m4 in range(4):
                for t in range(4):
                    units.append(lambda m4=m4, t=t: qk_unit(dma_k, KT, m4, t))
                for t in range(2):
                    units.append(lambda m4=m4, t=t: qk_unit(dma_q, QT, m4, t))
            return units

        def emit_norm(h):
            """Normalize head h's raw U by its softmax denominators."""
            rec = _recs.pop(h)
            for qc in range(2):
                bc = mm_psum.tile([64, 512], f32, tag="mm", name="bc")
                nc.tensor.matmul(
                    bc[:], ones_bc[64:65, :],
                    rec[64:65, qc * 512:(qc + 1) * 512],
                    start=True, stop=True)
                bc_sb = bc_pool.tile([64, 512], f32, tag="bc", name="bc_sb")
                nc.vector.tensor_copy(bc_sb[:], bc[:])
                nc.gpsimd.tensor_mul(
                    uraw[h][0:64, qc * 512:(qc + 1) * 512],
                    uraw[h][0:64, qc * 512:(qc + 1) * 512], bc_sb[:])

        _recs = {}

        def emit_attn(heads, QTs, KTs, VTs, uraw_pools, fillers,
                      hooks=None):
            """Attention for the given heads; filler units spread across
            k-steps. Normalize for head h is emitted one head late."""
            fillers = list(fillers)
            nfill = len(fillers)
            steps = len(heads) * 16
            done = 0
            for hh_i, h in enumerate(heads):
                if hooks and h in hooks:
                    hooks[h]()
                half = h // 8
                hh = h % 8
                QT, KT, VT = QTs[half], KTs[half], VTs[half]
                pair = hh // 2
                hb = (hh % 2) * 64
                Ups = u_psum.tile([65, 2, 512], f32, tag="up", name="up")
                for k in range(16):
                    sp = sp_psum.tile([128, 2, 512], f32, tag="sp", name="sp")
                    for qc in range(2):
                        nc.tensor.matmul(
                            sp[:, qc, :],
                            KT[pair][hb:hb + 64, k * 128:(k + 1) * 128],
                            QT[pair][hb:hb + 64, qc * 512:(qc + 1) * 512],
                            start=True, stop=True)
                    es = es_pool.tile([128, 2, 512], bf16, tag="es", name="es")
                    nc.scalar.activation(es[:], sp[:], EXP, scale=0.125)
                    for qc in range(2):
                        nc.tensor.matmul(
                            Ups[:, qc, :],
                            VT[k][:, hh, :],
                            es[:, qc, :],
                            start=(k == 0), stop=(k == 15))
                    done += 1
                    while fillers and (nfill - len(fillers)) * steps < done * nfill:
                        fillers.pop(0)()
                # free the PSUM slot fast: one copy stashes raw U + D
                ur = uraw_pools[half]().tile([65, NQ], bf16, tag=f"uraw{h}",
                                             name=f"uraw{h}")
                uraw[h] = ur
                nc.vector.tensor_copy(
                    ur[:].rearrange("p (a b) -> p a b", a=2), Ups[:, :, :])
                # slow reciprocal runs from SBUF, off the PSUM critical chain
                rec = rec_pool.tile([65, NQ], f16, tag="rec", name="rec")
                with nc.allow_low_precision(reason="softmax denom recip fp16"):
                    nc.vector.reciprocal(
                        rec[64:65, :], ur[64:65, :])
                _recs[h] = rec
                if hh_i > 0:
                    emit_norm(heads[hh_i - 1])
            emit_norm(heads[-1])
            for f in fillers:
                f()

        # ---------------- emission ----------------
        if True:
            xt_pool = tc.alloc_tile_pool(name="xt", bufs=1)
            w_pool = tc.alloc_tile_pool(name="w", bufs=2)
            xt = [xt_pool.tile([128, N], bf16, tag=f"xt{i}", name=f"xt{i}")
                  for i in range(8)]
            for i in range(8):
                nc.sync.dma_start(xt[i][:], xt_d.ap()[i * 128:(i + 1) * 128, :])

            qkv0 = tc.alloc_tile_pool(name="qkv0", bufs=1)
            QT0 = [qkv0.tile([128, NQ], bf16, tag=f"q{m}", name=f"q0{m}")
                   for m in range(4)]
            KT0 = [qkv0.tile([128, N], bf16, tag=f"k{m}", name=f"k0{m}")
                   for m in range(4)]
            VT0 = [qkv0.tile([128, 8, 65], bf16, tag=f"v{mk}", name=f"v0{mk}")
                   for mk in range(16)]
            p0_units = proj_units(0, w_pool, xt, QT0, KT0, VT0)
            # V + pair-0 K/Q serially (heads 0/1 cannot start without them;
            # Tile dependencies only look backward in emission order)
            for c in p0_units[:22]:
                c()
            p0_rest = p0_units[22:]

            qkv1 = tc.alloc_tile_pool(name="qkv1", bufs=1, side="right")
            QT1 = [qkv1.tile([128, NQ], bf16, tag=f"q{m}", name=f"q1{m}")
                   for m in range(4)]
            KT1 = [qkv1.tile([128, N], bf16, tag=f"k{m}", name=f"k1{m}")
                   for m in range(4)]
            VT1 = [qkv1.tile([128, 8, 65], bf16, tag=f"v{mk}", name=f"v1{mk}")
                   for mk in range(16)]
            p1_units = proj_units(1, w_pool, xt, QT1, KT1, VT1)

            state = {}

            def setup_b():
                qkv0.release()
                state["uraw_b"] = tc.alloc_tile_pool(name="uraw_b", bufs=1,
                                                     side="right")
                wo_pool = tc.alloc_tile_pool(name="wo", bufs=1, side="right")
                state["wo_pool"] = wo_pool
                state["WO"] = [wo_pool.tile([64, DIM], bf16, tag=f"wo{h}",
                                            name=f"wo{h}")
                               for h in range(HEADS)]
                for h in range(HEADS):
                    nc.sync.dma_start(state["WO"][h][:], wo_d.ap()[h])


            def setup_c():
                # xt and the w-block tiles are dead once proj(1) is done
                w_pool.release()
                xt_pool.release()
                state["st_pool"] = tc.alloc_tile_pool(name="st", bufs=2)
                state["FIN"] = [
                    state["st_pool"].tile([128, DIM], f32, tag=f"fin{qf}",
                                          name=f"fin{qf}", bufs=1)
                    for qf in range(8)]

            # pass A unit: heads 0-7 + bias for one qf -> resident FIN tile
            def passA(qf):
                WO = state["WO"]
                fin = state["FIN"][qf]
                for of in range(2):
                    ps = mm_psum.tile([128, 512], f32, tag="mm", name="mm")
                    for hh in range(8):
                        nc.tensor.matmul(
                            ps[:],
                            uraw[hh][0:64, qf * 128:(qf + 1) * 128],
                            WO[hh][:, of * 512:(of + 1) * 512],
                            start=(hh == 0), stop=False)
                    nc.tensor.matmul(
                        ps[:], ones_t[:, 0:128],
                        brow_t[:, of * 512:(of + 1) * 512],
                        start=False, stop=True)
                    nc.vector.tensor_copy(fin[:, of * 512:(of + 1) * 512],
                                          ps[:])

            uraw_pools = {0: lambda: uraw_a, 1: lambda: state["uraw_b"]}

            # heads 0-9: remaining proj0 + all proj1 units fill PE gaps
            emit_attn(range(0, 10), [QT0, QT1], [KT0, KT1], [VT0, VT1],
                      uraw_pools, p0_rest + p1_units[:34],
                      hooks={8: setup_b})
            # heads 10-15: pass A units fill PE gaps
            emit_attn(range(10, 16), [QT0, QT1], [KT0, KT1], [VT0, VT1],
                      uraw_pools,
                      p1_units[34:] +
                      [lambda qf=qf: passA(qf) for qf in range(8)],
                      hooks={10: setup_c})

            # pass B: heads 8-15 onto the resident partials -> out.
            # h15 (the norm-gated head) goes FIRST in each accumulation
            # group so the PE pays its normalize wait once, then streams.
            for qf in range(8):
                fin = state["FIN"][qf]
                for of in range(2):
                    ps = mm_psum.tile([128, 512], f32, tag="mm", name="mm")
                    for hh in [15] + list(range(8, 15)):
                        nc.tensor.matmul(
                            ps[:],
                            uraw[hh][0:64, qf * 128:(qf + 1) * 128],
                            state["WO"][hh][:, of * 512:(of + 1) * 512],
                            start=(hh == 15), stop=(hh == 14))
                    nc.vector.tensor_add(
                        fin[:, of * 512:(of + 1) * 512],
                        fin[:, of * 512:(of + 1) * 512], ps[:])
                nc.sync.dma_start(out_d.ap()[qf * 128:(qf + 1) * 128, :], fin[:])

            state["st_pool"].release()
            state["wo_pool"].release()
            state["uraw_b"].release()
            qkv1.release()

    nc.compile()
    return nc


def _get_nc():
    if "nc" not in _CACHE:
        _CACHE["nc"] = _build_nc()
    return _CACHE["nc"]


def _make_in_maps(x, w_qkv, w_out, b_out):
    bf = ml_dtypes.bfloat16
    wo = np.ascontiguousarray(w_out.reshape(HEADS, DH, DIM)).astype(bf)
    brow = np.asarray(b_out, np.float32).reshape(1, DIM).astype(bf)
    wqkv = np.ascontiguousarray(w_qkv, np.float32).astype(bf)
    in_maps = []
    for i in range(NCORES):
        b, half = i // 2, i % 2
        xt = np.asarray(x[b], np.float32).T.astype(bf)  # [DIM, N]
        if half:
            xt = np.concatenate([xt[:, NQ:], xt[:, :NQ]], axis=1)
        in_maps.append({
            "xt": np.ascontiguousarray(xt),
            "wqkv": wqkv,
            "wo": wo,
            "brow": brow,
        })
    return in_maps


def _assemble(results):
    out = np.empty((B, N, DIM), np.float32)
    for i in range(NCORES):
        b, half = i // 2, i % 2
        out[b, half * NQ:(half + 1) * NQ, :] = results[i]["out"]
    return out


def run(x, w_qkv, w_out, b_out, trace=False):
    """Run the kernel; returns (output, BassKernelResults)."""
    from concourse.bass_utils import run_bass_kernel_spmd
    nc = _get_nc()
    in_maps = _make_in_maps(x, w_qkv, w_out, b_out)
    res = run_bass_kernel_spmd(nc, in_maps, core_ids=list(range(NCORES)),
                               trace=trace)
    return _assemble(res.results), res


def kernel(x, w_qkv, w_out, b_out):
    out, _ = run(x, w_qkv, w_out, b_out, trace=False)
    return out



# revision 6
# speedup vs baseline: 1.2300x; 1.2300x over previous
"""Distributed multi-head attention kernel for 8 TRN2 NeuronCores.

Problem: x [4, 2048, 1024] -> qkv proj -> 16-head attention (d=64)
         -> out proj + bias -> [4, 2048, 1024].

Sharding (no collectives): core i handles batch b = i//2 and head-half
hh = i%2 (8 of the 16 heads, ALL 2048 query tokens). Each core projects
Q/K/V only for its own 8 heads (columns hh*512..hh*512+512 of each
block of w_qkv), runs attention for those heads over the full sequence,
and applies the out-projection restricted to its heads' rows of w_out.
The two cores of a batch produce additive partial outputs; the host
sums them and adds the bias.

Per-core pipeline (bf16 on the TensorE, fp32 PSUM accum), 8 local heads
= 4 pairs, each pair's two heads stacked on SBUF partitions 0:64 /
64:128 of the Q^T/K^T tiles:

  proj:  full-efficiency K=128 matmuls; V keeps a ones column per head
         so the PV matmul yields softmax denominators for free.
  attn:  per (pair, 512-query chunk, 128-key chunk):
           S^T via TWO row-tiled 64x128 matmuls (head0 on PE rows 0:63,
           head1 on rows 64:127 -- they execute CONCURRENTLY in the
           2x-row-tiled PE array), one exp() on the ScalarE over both
           heads' scores [128, 2, 512] (N=1024 per ACTIVATE), then two
           PV matmuls accumulating U^T[65, 512] per head (row 64 = the
           softmax denominator).
         The ScalarE exp stream is the critical resource (~294us); all
         projection and out-projection matmuls are interleaved into the
         PE's idle time underneath it.
  norm:  1/D via the fast DVE reciprocal, broadcast via a K=1 f32r
         matmul, multiply on the GpSimd engine straight into packed
         [128, 2048] per-pair tiles (so the out-proj contracts K=128).
  out:   pass A (pairs 0+1) runs as filler during pair 2/3 attention
         into resident f32 tiles; pass B (pairs 2+3) adds on top and
         streams to DRAM.
"""

import numpy as np
import ml_dtypes

B = 4
N = 2048
DIM = 1024
HEADS = 16
DH = 64
NCORES = 8
LHEADS = 8    # heads per core
PAIRS = 4     # head pairs per core

_CACHE = {}


def _build_nc():
    from contextlib import ExitStack

    import concourse.bass as bass
    import concourse.mybir as mybir
    import concourse.tile as tile
    from concourse import bacc

    f32 = mybir.dt.float32
    f32r = mybir.dt.float32r
    bf16 = mybir.dt.bfloat16
    EXP = mybir.ActivationFunctionType.Exp

    nc = bacc.Bacc("TRN2", target_bir_lowering=False, debug=False,
                   num_devices=NCORES)

    xt_d = nc.dram_tensor("xt", [DIM, N], bf16, kind="ExternalInput")
    # per-core slices of w_qkv: [DIM, 512] each for q, k, v
    wq_d = nc.dram_tensor("wq", [DIM, 512], bf16, kind="ExternalInput")
    wk_d = nc.dram_tensor("wk", [DIM, 512], bf16, kind="ExternalInput")
    wv_d = nc.dram_tensor("wv", [DIM, 512], bf16, kind="ExternalInput")
    wo_d = nc.dram_tensor("wo", [PAIRS, 128, DIM], bf16, kind="ExternalInput")
    out_d = nc.dram_tensor("out", [N, DIM], f32, kind="ExternalOutput")

    with tile.TileContext(nc) as tc, ExitStack() as top:
        const_pool = top.enter_context(tc.tile_pool(name="const", bufs=1))
        mm_psum = top.enter_context(tc.tile_pool(name="mmps", bufs=2, space="PSUM"))
        sp_psum = top.enter_context(tc.tile_pool(name="spps", bufs=2, space="PSUM"))
        u_psum = top.enter_context(tc.tile_pool(name="ups", bufs=2, space="PSUM"))
        es_pool = top.enter_context(tc.tile_pool(name="es", bufs=4))
        nrm_pool = top.enter_context(tc.tile_pool(name="nrm", bufs=4))
        upk_pool = top.enter_context(tc.tile_pool(name="upk", bufs=1))
        vt_pool = top.enter_context(tc.tile_pool(name="vt", bufs=1))
        wo_pool = top.enter_context(tc.tile_pool(name="wo", bufs=1))

        ones_f = const_pool.tile([1, 64], bf16, tag="ones_f", name="ones_f")
        nc.gpsimd.memset(ones_f[:], 1.0)

        # ---------------- DMA: v weights + xt(t0) first ----------------
        xt_pool = tc.alloc_tile_pool(name="xt", bufs=1)
        w_pool = tc.alloc_tile_pool(name="w", bufs=1)
        WQ = [w_pool.tile([128, 512], bf16, tag=f"wq{fc}", name=f"wq{fc}")
              for fc in range(8)]
        WK = [w_pool.tile([128, 512], bf16, tag=f"wk{fc}", name=f"wk{fc}")
              for fc in range(8)]
        WV = [w_pool.tile([128, 512], bf16, tag=f"wv{fc}", name=f"wv{fc}")
              for fc in range(8)]
        # xt as 8x4 tiles [128, 512]: xt[fc][t]
        xt = [[xt_pool.tile([128, 512], bf16, tag=f"xt{fc}_{t}",
                            name=f"xt{fc}_{t}") for t in range(4)]
              for fc in range(8)]

        for fc in range(8):
            nc.sync.dma_start(WV[fc][:], wv_d.ap()[fc * 128:(fc + 1) * 128, :])
        for fc in range(8):
            nc.sync.dma_start(xt[fc][0][:],
                              xt_d.ap()[fc * 128:(fc + 1) * 128, 0:512])
        for fc in range(8):
            nc.sync.dma_start(WK[fc][:], wk_d.ap()[fc * 128:(fc + 1) * 128, :])
        for fc in range(8):
            nc.sync.dma_start(xt[fc][1][:],
                              xt_d.ap()[fc * 128:(fc + 1) * 128, 512:1024])
        for fc in range(8):
            nc.sync.dma_start(WQ[fc][:], wq_d.ap()[fc * 128:(fc + 1) * 128, :])
        for t in range(2, 4):
            for fc in range(8):
                nc.sync.dma_start(xt[fc][t][:],
                                  xt_d.ap()[fc * 128:(fc + 1) * 128,
                                            t * 512:(t + 1) * 512])
        WO = [wo_pool.tile([128, DIM], bf16, tag=f"wo{p}", name=f"wo{p}")
              for p in range(PAIRS)]
        for p in range(PAIRS):
            nc.sync.dma_start(WO[p][:], wo_d.ap()[p])

        # ---------------- projection units ----------------
        qkv_pool = tc.alloc_tile_pool(name="qkv", bufs=1, side="right")
        QT = [qkv_pool.tile([128, N], bf16, tag=f"q{p}", name=f"q{p}")
              for p in range(PAIRS)]
        KT = [qkv_pool.tile([128, N], bf16, tag=f"k{p}", name=f"k{p}")
              for p in range(PAIRS)]
        VT = [vt_pool.tile([128, LHEADS, 65], bf16, tag=f"v{mk}",
                           name=f"v{mk}") for mk in range(16)]

        def v_unit(mk):
            ps = mm_psum.tile([128, 512], f32, tag="mm", name="mm")
            for fc in range(8):
                nc.tensor.matmul(
                    ps[:], xt[fc][mk // 4][:, (mk % 4) * 128:(mk % 4 + 1) * 128],
                    WV[fc][:], start=(fc == 0), stop=(fc == 7))
            nc.vector.tensor_copy(
                VT[mk][:, :, 0:64],
                ps[:].rearrange("p (h d) -> p h d", d=64))
            nc.gpsimd.memset(VT[mk][:, :, 64:65], 1.0)

        def qk_unit(wb, dest, p, t):
            ps = mm_psum.tile([128, 512], f32, tag="mm", name="mm")
            for fc in range(8):
                nc.tensor.matmul(
                    ps[:], wb[fc][:, p * 128:(p + 1) * 128],
                    xt[fc][t][:], start=(fc == 0), stop=(fc == 7))
            nc.vector.tensor_copy(dest[p][:, t * 512:(t + 1) * 512], ps[:])

        # ---------------- out-projection units ----------------
        state = {}

        def passA(tc_i, of):
            ps = mm_psum.tile([128, 512], f32, tag="mm", name="mm")
            for p in range(2):
                nc.tensor.matmul(
                    ps[:], state["UPK"][p][:, tc_i * 128:(tc_i + 1) * 128],
                    WO[p][:, of * 512:(of + 1) * 512],
                    start=(p == 0), stop=(p == 1))
            nc.vector.tensor_copy(
                state["FIN"][tc_i][:, of * 512:(of + 1) * 512], ps[:])

        def passB(tc_i, of):
            ps = mm_psum.tile([128, 512], f32, tag="mm", name="mm")
            for p in range(2, 4):
                nc.tensor.matmul(
                    ps[:], state["UPK"][p][:, tc_i * 128:(tc_i + 1) * 128],
                    WO[p][:, of * 512:(of + 1) * 512],
                    start=(p == 2), stop=(p == 3))
            fin = state["FIN"][tc_i]
            nc.vector.tensor_add(
                fin[:, of * 512:(of + 1) * 512],
                fin[:, of * 512:(of + 1) * 512], ps[:])
            if of == 1:
                nc.sync.dma_start(
                    out_d.ap()[tc_i * 128:(tc_i + 1) * 128, :], fin[:])

        # ---------------- attention ----------------
        UPK = [upk_pool.tile([128, N], bf16, tag=f"upk{p}", name=f"upk{p}")
               for p in range(PAIRS)]
        state["UPK"] = UPK

        def norm_unit(p, qc, h, u):
            """u [65, 512] PSUM: rows 0:64 raw U^T, row 64 denominator.
            Writes normalized bf16 into UPK[p][64h:64h+64, qc*512:...]."""
            # custom-DVE ops read from partition 0 of the AP's buffer, so
            # stage the denominator row into its own partition-0 tile first
            d_sb = nrm_pool.tile([1, 512], f32, tag="dsb", name="d_sb")
            nc.vector.tensor_copy(d_sb[:], u[64:65, :])
            rec = nrm_pool.tile([1, 512], f32, tag="rec", name="rec")
            nc.vector.reciprocal_approx_fast(rec[:], d_sb[:])
            rec_bf = nrm_pool.tile([1, 512], bf16, tag="recb", name="rec_bf")
            nc.vector.tensor_copy(rec_bf[:], rec[:])
            bc = mm_psum.tile([64, 512], f32, tag="mm", name="bc")
            nc.tensor.matmul(bc[:], ones_f[:], rec_bf[:],
                             start=True, stop=True)
            bc_sb = nrm_pool.tile([64, 512], f32, tag="bc", name="bc_sb")
            nc.vector.tensor_copy(bc_sb[:], bc[:])
            ur_sb = nrm_pool.tile([64, 512], bf16, tag="ur", name="ur_sb")
            nc.vector.tensor_copy(ur_sb[:], u[0:64, :])
            nc.gpsimd.tensor_mul(
                UPK[p][64 * h:64 * h + 64, qc * 512:(qc + 1) * 512],
                ur_sb[:], bc_sb[:])

        def attn_block(p, qc, fillers):
            """One (pair, query-chunk) softmax-attention block: 16 key
            chunks; fillers paced uniformly across them."""
            fillers = list(fillers)
            nfill = len(fillers)
            u0 = u_psum.tile([65, 512], f32, tag="u", name="u0")
            u1 = u_psum.tile([65, 512], f32, tag="u", name="u1")
            for k in range(16):
                sp = sp_psum.tile([128, 2, 512], f32, tag="sp", name="sp")
                nc.tensor.matmul(
                    sp[:, 0, :], KT[p][0:64, k * 128:(k + 1) * 128],
                    QT[p][0:64, qc * 512:(qc + 1) * 512],
                    start=True, stop=True)
                nc.tensor.matmul(
                    sp[:, 1, :], KT[p][64:128, k * 128:(k + 1) * 128],
                    QT[p][64:128, qc * 512:(qc + 1) * 512],
                    start=True, stop=True)
                es = es_pool.tile([128, 2, 512], bf16, tag="es", name="es")
                nc.scalar.activation(es[:], sp[:], EXP, scale=0.125)
                nc.tensor.matmul(u0[:], VT[k][:, 2 * p, :], es[:, 0, :],
                                 start=(k == 0), stop=(k == 15))
                nc.tensor.matmul(u1[:], VT[k][:, 2 * p + 1, :], es[:, 1, :],
                                 start=(k == 0), stop=(k == 15))
                while fillers and (nfill - len(fillers)) * 16 < (k + 1) * nfill:
                    fillers.pop(0)()
            for f in fillers:
                f()
            norm_unit(p, qc, 0, u0)
            norm_unit(p, qc, 1, u1)

        # ---------------- emission ----------------
        # prologue: V chunks 0-3, K pair0 (all t), Q pair0 t0
        for mk in range(4):
            v_unit(mk)
        for t in range(4):
            qk_unit(WK, KT, 0, t)
        qk_unit(WQ, QT, 0, 0)

        # filler schedules per (pair, qc) block
        sched = {
            (0, 0): [lambda: qk_unit(WQ, QT, 0, 1)] +
                    [lambda mk=mk: v_unit(mk) for mk in range(4, 16)],
        }
        rest = ([lambda: qk_unit(WQ, QT, 0, 2), lambda: qk_unit(WQ, QT, 0, 3)]
                + [lambda t=t: qk_unit(WK, KT, 1, t) for t in range(4)]
                + [lambda t=t: qk_unit(WQ, QT, 1, t) for t in range(4)]
                + [lambda t=t: qk_unit(WK, KT, 2, t) for t in range(4)]
                + [lambda t=t: qk_unit(WQ, QT, 2, t) for t in range(4)]
                + [lambda t=t: qk_unit(WK, KT, 3, t) for t in range(4)]
                + [lambda t=t: qk_unit(WQ, QT, 3, t) for t in range(4)])
        # spread `rest` over blocks (0,1)..(2,3): 11 blocks
        blocks = [(p, qc) for p in range(3) for qc in range(4)][1:]
        per = (len(rest) + len(blocks) - 1) // len(blocks)
        for i, blk in enumerate(blocks):
            sched[blk] = rest[i * per:(i + 1) * per]

        def setup_p3():
            # proj inputs are dead; make room and set up out-proj pass A
            w_pool.release()
            xt_pool.release()
            state["st_pool"] = tc.alloc_tile_pool(name="st", bufs=2)
            state["FIN"] = [
                state["st_pool"].tile([128, DIM], f32, tag=f"fin{i}",
                                      name=f"fin{i}", bufs=1)
                for i in range(16)]

        pA = [lambda i=i, of=of: passA(i, of)
              for i in range(16) for of in range(2)]
        sched[(3, 0)] = pA[:8]
        sched[(3, 1)] = pA[8:16]
        sched[(3, 2)] = pA[16:24]
        sched[(3, 3)] = pA[24:32]

        for p in range(PAIRS):
            for qc in range(4):
                if (p, qc) == (3, 0):
                    setup_p3()
                attn_block(p, qc, sched.get((p, qc), []))

        for i in range(16):
            for of in range(2):
                passB(i, of)

        state["st_pool"].release()
        qkv_pool.release()

    nc.compile()
    return nc


def _get_nc():
    if "nc" not in _CACHE:
        _CACHE["nc"] = _build_nc()
    return _CACHE["nc"]


def _make_in_maps(x, w_qkv, w_out, b_out):
    bf = ml_dtypes.bfloat16
    w_qkv = np.asarray(w_qkv, np.float32)
    w_out = np.asarray(w_out, np.float32)
    halves = []
    for hh in range(2):
        c0 = hh * 512
        wq = np.ascontiguousarray(w_qkv[:, c0:c0 + 512]).astype(bf)
        wk = np.ascontiguousarray(w_qkv[:, DIM + c0:DIM + c0 + 512]).astype(bf)
        wv = np.ascontiguousarray(
            w_qkv[:, 2 * DIM + c0:2 * DIM + c0 + 512]).astype(bf)
        wo = np.ascontiguousarray(
            w_out[c0:c0 + 512, :].reshape(PAIRS, 128, DIM)).astype(bf)
        halves.append((wq, wk, wv, wo))
    in_maps = []
    for i in range(NCORES):
        b, hh = i // 2, i % 2
        xt = np.ascontiguousarray(np.asarray(x[b], np.float32).T.astype(bf))
        wq, wk, wv, wo = halves[hh]
        in_maps.append({"xt": xt, "wq": wq, "wk": wk, "wv": wv, "wo": wo})
    return in_maps


def _assemble(results, b_out):
    out = np.empty((B, N, DIM), np.float32)
    bias = np.asarray(b_out, np.float32)
    for b in range(B):
        out[b] = results[2 * b]["out"] + results[2 * b + 1]["out"] + bias
    return out


def run(x, w_qkv, w_out, b_out, trace=False):
    """Run the kernel; returns (output, BassKernelResults)."""
    from concourse.bass_utils import run_bass_kernel_spmd
    nc = _get_nc()
    in_maps = _make_in_maps(x, w_qkv, w_out, b_out)
    res = run_bass_kernel_spmd(nc, in_maps, core_ids=list(range(NCORES)),
                               trace=trace)
    return _assemble(res.results, b_out), res


def kernel(x, w_qkv, w_out, b_out):
    out, _ = run(x, w_qkv, w_out, b_out, trace=False)
    return out


# revision 13
# speedup vs baseline: 1.3028x; 1.0592x over previous
"""Distributed multi-head attention kernel for 8 TRN2 NeuronCores.

Problem: x [4, 2048, 1024] -> qkv proj -> 16-head attention (d=64)
         -> out proj + bias -> [4, 2048, 1024].

Sharding (no collectives): core i handles batch b = i//2 and head-half
hh = i%2 (8 of the 16 heads, ALL 2048 query tokens). Each core projects
Q/K/V only for its own 8 heads (columns hh*512..hh*512+512 of each
block of w_qkv), runs attention for those heads over the full sequence,
and applies the out-projection restricted to its heads' rows of w_out.
The two cores of a batch produce additive partial outputs; the host
sums them and adds the bias.

Per-core pipeline (bf16 on the TensorE, fp32 PSUM accum), 8 local heads
= 4 pairs, each pair's two heads stacked on SBUF partitions 0:64 /
64:128 of the Q^T/K^T tiles:

  proj:  full-efficiency K=128 matmuls; V keeps a ones column per head
         so the PV matmul yields softmax denominators for free.
  attn:  per (pair, 512-query chunk, 128-key chunk):
           S^T via TWO row-tiled 64x128 matmuls (head0 on PE rows 0:63,
           head1 on rows 64:127 -- they execute CONCURRENTLY in the
           2x-row-tiled PE array), one exp() on the ScalarE over both
           heads' scores [128, 2, 512] (N=1024 per ACTIVATE), then two
           PV matmuls accumulating U^T[65, 512] per head (row 64 = the
           softmax denominator).
         The ScalarE exp stream is the critical resource (~294us); all
         projection and out-projection matmuls are interleaved into the
         PE's idle time underneath it.
  norm:  1/D via the fast DVE reciprocal, broadcast via a K=1 f32r
         matmul, multiply on the GpSimd engine straight into packed
         [128, 2048] per-pair tiles (so the out-proj contracts K=128).
  out:   pass A (pairs 0+1) runs as filler during pair 2/3 attention
         into resident f32 tiles; pass B (pairs 2+3) adds on top and
         streams to DRAM.
"""

import numpy as np
import ml_dtypes

B = 4
N = 2048
DIM = 1024
HEADS = 16
DH = 64
NCORES = 8
LHEADS = 8    # heads per core
PAIRS = 4     # head pairs per core

_CACHE = {}


def _build_nc():
    from contextlib import ExitStack

    import concourse.bass as bass
    import concourse.mybir as mybir
    import concourse.tile as tile
    from concourse import bacc

    f32 = mybir.dt.float32
    f16 = mybir.dt.float16
    bf16 = mybir.dt.bfloat16
    EXP = mybir.ActivationFunctionType.Exp

    nc = bacc.Bacc("TRN2", target_bir_lowering=False, debug=False,
                   num_devices=NCORES)

    xt_d = nc.dram_tensor("xt", [DIM, N], bf16, kind="ExternalInput")
    # per-core slices of w_qkv: [DIM, 512] each for q, k, v
    wq_d = nc.dram_tensor("wq", [DIM, 512], bf16, kind="ExternalInput")
    wk_d = nc.dram_tensor("wk", [DIM, 512], bf16, kind="ExternalInput")
    wv_d = nc.dram_tensor("wv", [DIM, 512], bf16, kind="ExternalInput")
    wo_d = nc.dram_tensor("wo", [PAIRS, 128, DIM], bf16, kind="ExternalInput")
    out_d = nc.dram_tensor("out", [N, DIM], f16, kind="ExternalOutput")

    with tile.TileContext(nc) as tc, ExitStack() as top:
        const_pool = top.enter_context(tc.tile_pool(name="const", bufs=1))
        mm_psum = top.enter_context(tc.tile_pool(name="mmps", bufs=2, space="PSUM"))
        sp_psum = top.enter_context(tc.tile_pool(name="spps", bufs=2, space="PSUM"))
        u_psum = top.enter_context(tc.tile_pool(name="ups", bufs=2, space="PSUM"))
        es_pool = top.enter_context(tc.tile_pool(name="es", bufs=4))
        nrm_pool = top.enter_context(tc.tile_pool(name="nrm", bufs=4))
        upk_pool = top.enter_context(tc.tile_pool(name="upk", bufs=1))
        vt_pool = top.enter_context(tc.tile_pool(name="vt", bufs=1))
        wo_pool = top.enter_context(tc.tile_pool(name="wo", bufs=1))

        ones_f = const_pool.tile([1, 64], bf16, tag="ones_f", name="ones_f")
        nc.gpsimd.memset(ones_f[:], 1.0)

        # ---------------- DMA: v weights + xt(t0) first ----------------
        xt_pool = tc.alloc_tile_pool(name="xt", bufs=1)
        w_pool = tc.alloc_tile_pool(name="w", bufs=1)
        WQ = [w_pool.tile([128, 512], bf16, tag=f"wq{fc}", name=f"wq{fc}")
              for fc in range(8)]
        WK = [w_pool.tile([128, 512], bf16, tag=f"wk{fc}", name=f"wk{fc}")
              for fc in range(8)]
        WV = [w_pool.tile([128, 512], bf16, tag=f"wv{fc}", name=f"wv{fc}")
              for fc in range(8)]
        # xt as 8x4 tiles [128, 512]: xt[fc][t]
        xt = [[xt_pool.tile([128, 512], bf16, tag=f"xt{fc}_{t}",
                            name=f"xt{fc}_{t}") for t in range(4)]
              for fc in range(8)]

        # weights on the scalar-engine HWDGE queue (ACT idle in prologue),
        # activations on the sync-engine queue -- two DMA streams in parallel
        for fc in range(8):
            nc.scalar.dma_start(WV[fc][:], wv_d.ap()[fc * 128:(fc + 1) * 128, :])
        for fc in range(8):
            nc.sync.dma_start(xt[fc][0][:],
                              xt_d.ap()[fc * 128:(fc + 1) * 128, 0:512])
        for fc in range(8):
            nc.scalar.dma_start(WK[fc][:], wk_d.ap()[fc * 128:(fc + 1) * 128, :])
        for fc in range(8):
            nc.sync.dma_start(xt[fc][1][:],
                              xt_d.ap()[fc * 128:(fc + 1) * 128, 512:1024])
        for fc in range(8):
            nc.scalar.dma_start(WQ[fc][:], wq_d.ap()[fc * 128:(fc + 1) * 128, :])
        for t in range(2, 4):
            for fc in range(8):
                nc.sync.dma_start(xt[fc][t][:],
                                  xt_d.ap()[fc * 128:(fc + 1) * 128,
                                            t * 512:(t + 1) * 512])
        WO = [wo_pool.tile([128, DIM], bf16, tag=f"wo{p}", name=f"wo{p}")
              for p in range(PAIRS)]
        for p in range(PAIRS):
            nc.scalar.dma_start(WO[p][:], wo_d.ap()[p])

        # ---------------- projection units ----------------
        qkv_pool = tc.alloc_tile_pool(name="qkv", bufs=1, side="right")
        QT = [qkv_pool.tile([128, N], bf16, tag=f"q{p}", name=f"q{p}")
              for p in range(PAIRS)]
        KT = [qkv_pool.tile([128, N], bf16, tag=f"k{p}", name=f"k{p}")
              for p in range(PAIRS)]
        VT = [vt_pool.tile([128, LHEADS, 65], bf16, tag=f"v{mk}",
                           name=f"v{mk}") for mk in range(16)]

        def v_unit(mk):
            ps = mm_psum.tile([128, 512], f32, tag="mm", name="mm")
            for fc in range(8):
                nc.tensor.matmul(
                    ps[:], xt[fc][mk // 4][:, (mk % 4) * 128:(mk % 4 + 1) * 128],
                    WV[fc][:], start=(fc == 0), stop=(fc == 7))
            nc.vector.tensor_copy(
                VT[mk][:, :, 0:64],
                ps[:].rearrange("p (h d) -> p h d", d=64))
            nc.gpsimd.memset(VT[mk][:, :, 64:65], 1.0)

        def qk_unit(wb, dest, p, t):
            ps = mm_psum.tile([128, 512], f32, tag="mm", name="mm")
            for fc in range(8):
                nc.tensor.matmul(
                    ps[:], wb[fc][:, p * 128:(p + 1) * 128],
                    xt[fc][t][:], start=(fc == 0), stop=(fc == 7))
            nc.vector.tensor_copy(dest[p][:, t * 512:(t + 1) * 512], ps[:])

        # ---------------- out-projection units ----------------
        state = {}

        def passA(tc_i, of):
            ps = mm_psum.tile([128, 512], f32, tag="mm", name="mm")
            for p in range(2):
                nc.tensor.matmul(
                    ps[:], state["UPK"][p][:, tc_i * 128:(tc_i + 1) * 128],
                    WO[p][:, of * 512:(of + 1) * 512],
                    start=(p == 0), stop=(p == 1))
            nc.vector.tensor_copy(
                state["FIN"][tc_i][:, of * 512:(of + 1) * 512], ps[:])

        def passB(tc_i, of):
            ps = mm_psum.tile([128, 512], f32, tag="mm", name="mm")
            for p in range(2, 4):
                nc.tensor.matmul(
                    ps[:], state["UPK"][p][:, tc_i * 128:(tc_i + 1) * 128],
                    WO[p][:, of * 512:(of + 1) * 512],
                    start=(p == 2), stop=(p == 3))
            fin = state["FIN"][tc_i]
            with nc.allow_low_precision(reason="f16 partial output"):
                nc.vector.tensor_add(
                    fin[:, of * 512:(of + 1) * 512],
                    fin[:, of * 512:(of + 1) * 512], ps[:])
            if of == 1:
                eng = nc.sync if tc_i % 2 == 0 else nc.scalar
                eng.dma_start(
                    out_d.ap()[tc_i * 128:(tc_i + 1) * 128, :], fin[:])

        # ---------------- attention ----------------
        UPK = [upk_pool.tile([128, N], bf16, tag=f"upk{p}", name=f"upk{p}")
               for p in range(PAIRS)]
        state["UPK"] = UPK

        def norm_unit(p, qc, h, u):
            """u [65, 512] PSUM: rows 0:64 raw U^T, row 64 denominator.
            Writes normalized bf16 into UPK[p][64h:64h+64, qc*512:...]."""
            # drain the PSUM tile first so the next block's PV can claim it
            ur_sb = nrm_pool.tile([64, 512], bf16, tag="ur", name="ur_sb")
            nc.vector.tensor_copy(ur_sb[:], u[0:64, :])
            # custom-DVE ops read from partition 0 of the AP's buffer, so
            # stage the denominator row into its own partition-0 tile first
            d_sb = nrm_pool.tile([1, 512], f32, tag="dsb", name="d_sb")
            nc.vector.tensor_copy(d_sb[:], u[64:65, :])
            rec = nrm_pool.tile([1, 512], f32, tag="rec", name="rec")
            nc.vector.reciprocal_approx_fast(rec[:], d_sb[:])
            rec_bf = nrm_pool.tile([1, 512], bf16, tag="recb", name="rec_bf")
            nc.vector.tensor_copy(rec_bf[:], rec[:])
            bc = mm_psum.tile([64, 512], f32, tag="mm", name="bc")
            nc.tensor.matmul(bc[:], ones_f[:], rec_bf[:],
                             start=True, stop=True)
            bc_sb = nrm_pool.tile([64, 512], f32, tag="bc", name="bc_sb")
            nc.vector.tensor_copy(bc_sb[:], bc[:])
            nc.gpsimd.tensor_mul(
                UPK[p][64 * h:64 * h + 64, qc * 512:(qc + 1) * 512],
                ur_sb[:], bc_sb[:])

        def attn_block(p, qc, fillers):
            """One (pair, query-chunk) softmax-attention block: 16 key
            chunks; fillers paced uniformly across them."""
            fillers = list(fillers)
            nfill = len(fillers)
            u0 = u_psum.tile([65, 512], f32, tag="u", name="u0")
            u1 = u_psum.tile([65, 512], f32, tag="u", name="u1")
            for k in range(16):
                sp = sp_psum.tile([128, 2, 512], f32, tag="sp", name="sp")
                nc.tensor.matmul(
                    sp[:, 0, :], KT[p][0:64, k * 128:(k + 1) * 128],
                    QT[p][0:64, qc * 512:(qc + 1) * 512],
                    start=True, stop=True)
                nc.tensor.matmul(
                    sp[:, 1, :], KT[p][64:128, k * 128:(k + 1) * 128],
                    QT[p][64:128, qc * 512:(qc + 1) * 512],
                    start=True, stop=True)
                es = es_pool.tile([128, 2, 512], bf16, tag="es", name="es")
                nc.scalar.activation(es[:], sp[:], EXP, scale=0.125)
                nc.tensor.matmul(u0[:], VT[k][:, 2 * p, :], es[:, 0, :],
                                 start=(k == 0), stop=(k == 15))
                nc.tensor.matmul(u1[:], VT[k][:, 2 * p + 1, :], es[:, 1, :],
                                 start=(k == 0), stop=(k == 15))
                while fillers and (nfill - len(fillers)) * 16 < (k + 1) * nfill:
                    fillers.pop(0)()
            for f in fillers:
                f()
            norm_unit(p, qc, 0, u0)
            norm_unit(p, qc, 1, u1)

        # ---------------- emission ----------------
        # prologue: V chunks 0-3, K pair0 (all t), Q pair0 t0
        for mk in range(4):
            v_unit(mk)
        for t in range(4):
            qk_unit(WK, KT, 0, t)
        qk_unit(WQ, QT, 0, 0)

        # filler schedules per (pair, qc) block
        sched = {
            (0, 0): [lambda: qk_unit(WQ, QT, 0, 1)] +
                    [lambda mk=mk: v_unit(mk) for mk in range(4, 16)],
        }
        rest = ([lambda: qk_unit(WQ, QT, 0, 2), lambda: qk_unit(WQ, QT, 0, 3)]
                + [lambda t=t: qk_unit(WK, KT, 1, t) for t in range(4)]
                + [lambda t=t: qk_unit(WQ, QT, 1, t) for t in range(4)]
                + [lambda t=t: qk_unit(WK, KT, 2, t) for t in range(4)]
                + [lambda t=t: qk_unit(WQ, QT, 2, t) for t in range(4)]
                + [lambda t=t: qk_unit(WK, KT, 3, t) for t in range(4)]
                + [lambda t=t: qk_unit(WQ, QT, 3, t) for t in range(4)])
        # spread `rest` over blocks (0,1)..(2,3): 11 blocks
        blocks = [(p, qc) for p in range(3) for qc in range(4)][1:]
        per = (len(rest) + len(blocks) - 1) // len(blocks)
        for i, blk in enumerate(blocks):
            sched[blk] = rest[i * per:(i + 1) * per]

        def setup_p3():
            # proj inputs are dead; make room and set up out-proj pass A
            w_pool.release()
            xt_pool.release()
            state["st_pool"] = tc.alloc_tile_pool(name="st", bufs=2)
            state["FIN"] = [
                state["st_pool"].tile([128, DIM], f16, tag=f"fin{i}",
                                      name=f"fin{i}", bufs=1)
                for i in range(16)]

        pA = [lambda i=i, of=of: passA(i, of)
              for i in range(16) for of in range(2)]
        sched[(3, 0)] = pA[:8]
        sched[(3, 1)] = pA[8:16]
        sched[(3, 2)] = pA[16:24]
        sched[(3, 3)] = pA[24:32]

        for p in range(PAIRS):
            for qc in range(4):
                if (p, qc) == (3, 0):
                    setup_p3()
                attn_block(p, qc, sched.get((p, qc), []))

        for i in range(16):
            for of in range(2):
                passB(i, of)

        state["st_pool"].release()
        qkv_pool.release()

    nc.compile()
    return nc


def _get_nc():
    if "nc" not in _CACHE:
        _CACHE["nc"] = _build_nc()
    return _CACHE["nc"]


def _make_in_maps(x, w_qkv, w_out, b_out):
    bf = ml_dtypes.bfloat16
    w_qkv = np.asarray(w_qkv, np.float32)
    w_out = np.asarray(w_out, np.float32)
    halves = []
    for hh in range(2):
        c0 = hh * 512
        wq = np.ascontiguousarray(w_qkv[:, c0:c0 + 512]).astype(bf)
        wk = np.ascontiguousarray(w_qkv[:, DIM + c0:DIM + c0 + 512]).astype(bf)
        wv = np.ascontiguousarray(
            w_qkv[:, 2 * DIM + c0:2 * DIM + c0 + 512]).astype(bf)
        wo = np.ascontiguousarray(
            w_out[c0:c0 + 512, :].reshape(PAIRS, 128, DIM)).astype(bf)
        halves.append((wq, wk, wv, wo))
    in_maps = []
    for i in range(NCORES):
        b, hh = i // 2, i % 2
        xt = np.ascontiguousarray(np.asarray(x[b], np.float32).T.astype(bf))
        wq, wk, wv, wo = halves[hh]
        in_maps.append({"xt": xt, "wq": wq, "wk": wk, "wv": wv, "wo": wo})
    return in_maps


def _assemble(results, b_out):
    out = np.empty((B, N, DIM), np.float32)
    bias = np.asarray(b_out, np.float32)
    for b in range(B):
        out[b] = (np.asarray(results[2 * b]["out"], np.float32)
                  + np.asarray(results[2 * b + 1]["out"], np.float32) + bias)
    return out


def run(x, w_qkv, w_out, b_out, trace=False):
    """Run the kernel; returns (output, BassKernelResults)."""
    from concourse.bass_utils import run_bass_kernel_spmd
    nc = _get_nc()
    in_maps = _make_in_maps(x, w_qkv, w_out, b_out)
    res = run_bass_kernel_spmd(nc, in_maps, core_ids=list(range(NCORES)),
                               trace=trace)
    return _assemble(res.results, b_out), res


def kernel(x, w_qkv, w_out, b_out):
    out, _ = run(x, w_qkv, w_out, b_out, trace=False)
    return out


# revision 19
# speedup vs baseline: 1.3717x; 1.0529x over previous
"""Distributed multi-head attention kernel for 8 TRN2 NeuronCores.

Problem: x [4, 2048, 1024] -> qkv proj -> 16-head attention (d=64)
         -> out proj + bias -> [4, 2048, 1024].

Sharding (no collectives): core i handles batch b = i//2 and head-half
hh = i%2 (8 of the 16 heads, ALL 2048 query tokens). Each core projects
Q/K/V only for its own 8 heads (columns hh*512..hh*512+512 of each
block of w_qkv), runs attention for those heads over the full sequence,
and applies the out-projection restricted to its heads' rows of w_out.
The two cores of a batch produce additive partial outputs; the host
sums them and adds the bias.

Per-core pipeline (bf16 on the TensorE, fp32 PSUM accum), 8 local heads
= 4 pairs, each pair's two heads stacked on SBUF partitions 0:64 /
64:128 of the Q^T/K^T tiles:

  proj:  full-efficiency K=128 matmuls; V keeps a ones column per head
         so the PV matmul yields softmax denominators for free.
  attn:  per (pair, 512-query chunk, 128-key chunk):
           S^T via TWO row-tiled 64x128 matmuls (head0 on PE rows 0:63,
           head1 on rows 64:127 -- they execute CONCURRENTLY in the
           2x-row-tiled PE array), one exp() on the ScalarE over both
           heads' scores [128, 2, 512] (N=1024 per ACTIVATE), then two
           PV matmuls accumulating U^T[65, 512] per head (row 64 = the
           softmax denominator).
         The ScalarE exp stream is the critical resource (~294us); all
         projection and out-projection matmuls are interleaved into the
         PE's idle time underneath it.
  norm:  1/D via the fast DVE reciprocal, broadcast via a K=1 f32r
         matmul, multiply on the GpSimd engine straight into packed
         [128, 2048] per-pair tiles (so the out-proj contracts K=128).
  out:   pass A (pairs 0+1) runs as filler during pair 2/3 attention
         into resident f32 tiles; pass B (pairs 2+3) adds on top and
         streams to DRAM.
"""

import numpy as np
import ml_dtypes

B = 4
N = 2048
DIM = 1024
HEADS = 16
DH = 64
NCORES = 8
LHEADS = 8    # heads per core
PAIRS = 4     # head pairs per core

_CACHE = {}


def _build_nc():
    from contextlib import ExitStack

    import concourse.bass as bass
    import concourse.mybir as mybir
    import concourse.tile as tile
    from concourse import bacc

    f32 = mybir.dt.float32
    f16 = mybir.dt.float16
    bf16 = mybir.dt.bfloat16
    EXP = mybir.ActivationFunctionType.Exp

    nc = bacc.Bacc("TRN2", target_bir_lowering=False, debug=False,
                   num_devices=NCORES)

    xt_d = nc.dram_tensor("xt", [DIM, N], bf16, kind="ExternalInput")
    # per-core slices of w_qkv: [DIM, 512] each for q, k, v
    wq_d = nc.dram_tensor("wq", [DIM, 512], bf16, kind="ExternalInput")
    wk_d = nc.dram_tensor("wk", [DIM, 512], bf16, kind="ExternalInput")
    wv_d = nc.dram_tensor("wv", [DIM, 512], bf16, kind="ExternalInput")
    wo_d = nc.dram_tensor("wo", [PAIRS, 128, DIM], bf16, kind="ExternalInput")
    out_d = nc.dram_tensor("out", [N, DIM], f16, kind="ExternalOutput")

    with tile.TileContext(nc) as tc, ExitStack() as top:
        const_pool = top.enter_context(tc.tile_pool(name="const", bufs=1))
        mm_psum = top.enter_context(tc.tile_pool(name="mmps", bufs=2, space="PSUM"))
        sp_psum = top.enter_context(tc.tile_pool(name="spps", bufs=2, space="PSUM"))
        u_psum = top.enter_context(tc.tile_pool(name="ups", bufs=2, space="PSUM"))
        es_pool = top.enter_context(tc.tile_pool(name="es", bufs=4))
        nrm_pool = top.enter_context(tc.tile_pool(name="nrm", bufs=4))
        upk_pool = top.enter_context(tc.tile_pool(name="upk", bufs=1))
        vt_pool = top.enter_context(tc.tile_pool(name="vt", bufs=1))
        wo_pool = top.enter_context(tc.tile_pool(name="wo", bufs=1))

        ones_f = const_pool.tile([1, 64], bf16, tag="ones_f", name="ones_f")
        nc.gpsimd.memset(ones_f[:], 1.0)

        # ---------------- DMA: v weights + xt(t0) first ----------------
        xt_pool = tc.alloc_tile_pool(name="xt", bufs=1)
        w_pool = tc.alloc_tile_pool(name="w", bufs=1)
        # weights as single multi-slot tiles: [128 part, fc, cols] -- each
        # loads with ONE descriptor (the ~600ns/DMA cost is fixed overhead)
        WQ = w_pool.tile([128, 8, 512], bf16, tag="wq", name="wq")
        WK = w_pool.tile([128, 8, 512], bf16, tag="wk", name="wk")
        WV = w_pool.tile([128, 8, 512], bf16, tag="wv", name="wv")
        # xt split into the t=0 token chunk (unblocks V/K/Q pair-0 fast)
        # and the rest; separate tiles so dependencies don't conflate them
        XT0 = xt_pool.tile([128, 8, 512], bf16, tag="xt0", name="xt0")
        XT1 = xt_pool.tile([128, 8, 1536], bf16, tag="xt1", name="xt1")

        def xt_ap(fc, lo, width):
            """AP for xt[fc*128:(fc+1)*128, lo:lo+width] (token columns)."""
            if lo + width <= 512:
                return XT0[:, fc, lo:lo + width]
            assert lo >= 512
            return XT1[:, fc, lo - 512:lo - 512 + width]

        # weights on the scalar-engine HWDGE queue (ACT idle in prologue),
        # activations on the sync-engine queue -- two DMA streams in parallel
        xt_r = xt_d.ap().rearrange("(f p) c -> p f c", p=128)
        nc.scalar.dma_start(WV[:], wv_d.ap().rearrange("(f p) c -> p f c", p=128))
        nc.sync.dma_start(XT0[:], xt_r[:, :, 0:512])
        nc.scalar.dma_start(WK[:], wk_d.ap().rearrange("(f p) c -> p f c", p=128))
        nc.scalar.dma_start(WQ[:], wq_d.ap().rearrange("(f p) c -> p f c", p=128))
        nc.sync.dma_start(XT1[:], xt_r[:, :, 512:2048])
        WO = [wo_pool.tile([128, DIM], bf16, tag=f"wo{p}", name=f"wo{p}")
              for p in range(PAIRS)]
        for p in range(PAIRS):
            nc.scalar.dma_start(WO[p][:], wo_d.ap()[p])

        # ---------------- projection units ----------------
        qkv_pool = tc.alloc_tile_pool(name="qkv", bufs=1, side="right")
        QT = [qkv_pool.tile([128, N], bf16, tag=f"q{p}", name=f"q{p}")
              for p in range(PAIRS)]
        KT = [qkv_pool.tile([128, N], bf16, tag=f"k{p}", name=f"k{p}")
              for p in range(PAIRS)]
        VT = [vt_pool.tile([128, LHEADS, 65], bf16, tag=f"v{mk}",
                           name=f"v{mk}") for mk in range(16)]

        def v_unit(mk):
            ps = mm_psum.tile([128, 512], f32, tag="mm", name="mm")
            for fc in range(8):
                nc.tensor.matmul(
                    ps[:], xt_ap(fc, mk * 128, 128),
                    WV[:, fc, :], start=(fc == 0), stop=(fc == 7))
            nc.vector.tensor_copy(
                VT[mk][:, :, 0:64],
                ps[:].rearrange("p (h d) -> p h d", d=64))
            nc.gpsimd.memset(VT[mk][:, :, 64:65], 1.0)

        def qk_unit(wb, dest, p, t):
            ps = mm_psum.tile([128, 512], f32, tag="mm", name="mm")
            for fc in range(8):
                nc.tensor.matmul(
                    ps[:], wb[:, fc, p * 128:(p + 1) * 128],
                    xt_ap(fc, t * 512, 512), start=(fc == 0), stop=(fc == 7))
            nc.vector.tensor_copy(dest[p][:, t * 512:(t + 1) * 512], ps[:])

        # ---------------- out-projection units ----------------
        state = {}

        def passA(tc_i, of):
            ps = mm_psum.tile([128, 512], f32, tag="mm", name="mm")
            for p in range(2):
                nc.tensor.matmul(
                    ps[:], state["UPK"][p][:, tc_i * 128:(tc_i + 1) * 128],
                    WO[p][:, of * 512:(of + 1) * 512],
                    start=(p == 0), stop=(p == 1))
            nc.vector.tensor_copy(
                state["FIN"][tc_i][:, of * 512:(of + 1) * 512], ps[:])

        def passB(tc_i, of, can_use_scalar_q=False):
            ps = mm_psum.tile([128, 512], f32, tag="mm", name="mm")
            for p in range(2, 4):
                nc.tensor.matmul(
                    ps[:], state["UPK"][p][:, tc_i * 128:(tc_i + 1) * 128],
                    WO[p][:, of * 512:(of + 1) * 512],
                    start=(p == 2), stop=(p == 3))
            fin = state["FIN"][tc_i]
            with nc.allow_low_precision(reason="f16 partial output"):
                nc.vector.tensor_add(
                    fin[:, of * 512:(of + 1) * 512],
                    fin[:, of * 512:(of + 1) * 512], ps[:])
            if of == 1:
                # while exp is still streaming, the scalar HWDGE queue would
                # insert DMA bubbles into the ACTIVATE stream -- sync only
                eng = nc.scalar if (can_use_scalar_q and tc_i % 2) else nc.sync
                eng.dma_start(
                    out_d.ap()[tc_i * 128:(tc_i + 1) * 128, :], fin[:])

        # ---------------- attention ----------------
        UPK = [upk_pool.tile([128, N], bf16, tag=f"upk{p}", name=f"upk{p}")
               for p in range(PAIRS)]
        state["UPK"] = UPK

        def norm_drain(p, qc, h, u):
            """DVE-only drain of the PSUM accumulator (frees the banks for
            the next block). Returns args for norm_finish."""
            ur_sb = nrm_pool.tile([64, 512], bf16, tag="ur", name="ur_sb")
            nc.vector.tensor_copy(ur_sb[:], u[0:64, :])
            # custom-DVE ops read from partition 0 of the AP's buffer, so
            # stage the denominator row into its own partition-0 tile first
            d_sb = nrm_pool.tile([1, 512], f32, tag="dsb", name="d_sb")
            nc.vector.tensor_copy(d_sb[:], u[64:65, :])
            return (p, qc, h, ur_sb, d_sb)

        def norm_finish(p, qc, h, ur_sb, d_sb):
            """Reciprocal + broadcast + multiply. Deferred into the next
            block so the bc matmul never makes the PE wait on the DVE."""
            rec = nrm_pool.tile([1, 512], f32, tag="rec", name="rec")
            nc.vector.reciprocal_approx_fast(rec[:], d_sb[:])
            rec_bf = nrm_pool.tile([1, 512], bf16, tag="recb", name="rec_bf")
            nc.vector.tensor_copy(rec_bf[:], rec[:])
            bc = mm_psum.tile([64, 512], f32, tag="mm", name="bc")
            nc.tensor.matmul(bc[:], ones_f[:], rec_bf[:],
                             start=True, stop=True)
            bc_sb = nrm_pool.tile([64, 512], f32, tag="bc", name="bc_sb")
            nc.vector.tensor_copy(bc_sb[:], bc[:])
            nc.gpsimd.tensor_mul(
                UPK[p][64 * h:64 * h + 64, qc * 512:(qc + 1) * 512],
                ur_sb[:], bc_sb[:])

        pending = []  # norm_finish args deferred from the previous block

        def attn_block(p, qc, fillers):
            """One (pair, query-chunk) softmax-attention block: 16 key
            chunks; fillers paced uniformly across them."""
            fillers = list(fillers)
            nfill = len(fillers)
            u0 = u_psum.tile([65, 512], f32, tag="u", name="u0")
            u1 = u_psum.tile([65, 512], f32, tag="u", name="u1")
            for k in range(16):
                sp = sp_psum.tile([128, 2, 512], f32, tag="sp", name="sp")
                nc.tensor.matmul(
                    sp[:, 0, :], KT[p][0:64, k * 128:(k + 1) * 128],
                    QT[p][0:64, qc * 512:(qc + 1) * 512],
                    start=True, stop=True)
                nc.tensor.matmul(
                    sp[:, 1, :], KT[p][64:128, k * 128:(k + 1) * 128],
                    QT[p][64:128, qc * 512:(qc + 1) * 512],
                    start=True, stop=True)
                es = es_pool.tile([128, 2, 512], bf16, tag="es", name="es")
                nc.scalar.activation(es[:], sp[:], EXP, scale=0.125)
                nc.tensor.matmul(u0[:], VT[k][:, 2 * p, :], es[:, 0, :],
                                 start=(k == 0), stop=(k == 15))
                nc.tensor.matmul(u1[:], VT[k][:, 2 * p + 1, :], es[:, 1, :],
                                 start=(k == 0), stop=(k == 15))
                if k == 1:
                    while pending:
                        norm_finish(*pending.pop(0))
                while fillers and (nfill - len(fillers)) * 16 < (k + 1) * nfill:
                    fillers.pop(0)()
            for f in fillers:
                f()
            pending.append(norm_drain(p, qc, 0, u0))
            pending.append(norm_drain(p, qc, 1, u1))

        # ---------------- emission ----------------
        # prologue: V chunks 0-3, K pair0 (all t), Q pair0 t0
        for mk in range(4):
            v_unit(mk)
        for t in range(4):
            qk_unit(WK, KT, 0, t)
        qk_unit(WQ, QT, 0, 0)

        # filler schedules per (pair, qc) block
        sched = {
            (0, 0): [lambda: qk_unit(WQ, QT, 0, 1)] +
                    [lambda mk=mk: v_unit(mk) for mk in range(4, 16)],
        }
        rest = ([lambda: qk_unit(WQ, QT, 0, 2), lambda: qk_unit(WQ, QT, 0, 3)]
                + [lambda t=t: qk_unit(WK, KT, 1, t) for t in range(4)]
                + [lambda t=t: qk_unit(WQ, QT, 1, t) for t in range(4)]
                + [lambda t=t: qk_unit(WK, KT, 2, t) for t in range(4)]
                + [lambda t=t: qk_unit(WQ, QT, 2, t) for t in range(4)]
                + [lambda t=t: qk_unit(WK, KT, 3, t) for t in range(4)]
                + [lambda t=t: qk_unit(WQ, QT, 3, t) for t in range(4)])
        # spread `rest` over blocks (0,1)..(2,3): 11 blocks
        blocks = [(p, qc) for p in range(3) for qc in range(4)][1:]
        per = (len(rest) + len(blocks) - 1) // len(blocks)
        for i, blk in enumerate(blocks):
            sched[blk] = rest[i * per:(i + 1) * per]

        def setup_p3():
            # proj inputs are dead; make room and set up out-proj pass A
            w_pool.release()
            xt_pool.release()
            state["st_pool"] = tc.alloc_tile_pool(name="st", bufs=2)
            state["FIN"] = [
                state["st_pool"].tile([128, DIM], f16, tag=f"fin{i}",
                                      name=f"fin{i}", bufs=1)
                for i in range(16)]

        pA = [lambda i=i, of=of: passA(i, of)
              for i in range(16) for of in range(2)]
        pB = [lambda i=i, of=of: passB(i, of)
              for i in range(12) for of in range(2)]
        sched[(3, 0)] = pA[:16]
        sched[(3, 1)] = pA[16:32]
        sched[(3, 2)] = pB[:8]     # tc 0-3  (needs pair-3 qc0 norm)
        sched[(3, 3)] = pB[8:24]   # tc 4-11 (needs pair-3 qc1/qc2 norms)

        for p in range(PAIRS):
            for qc in range(4):
                if (p, qc) == (3, 0):
                    setup_p3()
                attn_block(p, qc, sched.get((p, qc), []))

        while pending:
            norm_finish(*pending.pop(0))
        for i in range(12, 16):
            for of in range(2):
                passB(i, of, can_use_scalar_q=True)

        state["st_pool"].release()
        qkv_pool.release()

    nc.compile()
    return nc


def _get_nc():
    if "nc" not in _CACHE:
        _CACHE["nc"] = _build_nc()
    return _CACHE["nc"]


def _make_in_maps(x, w_qkv, w_out, b_out):
    bf = ml_dtypes.bfloat16
    w_qkv = np.asarray(w_qkv, np.float32)
    w_out = np.asarray(w_out, np.float32)
    halves = []
    for hh in range(2):
        c0 = hh * 512
        wq = np.ascontiguousarray(w_qkv[:, c0:c0 + 512]).astype(bf)
        wk = np.ascontiguousarray(w_qkv[:, DIM + c0:DIM + c0 + 512]).astype(bf)
        wv = np.ascontiguousarray(
            w_qkv[:, 2 * DIM + c0:2 * DIM + c0 + 512]).astype(bf)
        wo = np.ascontiguousarray(
            w_out[c0:c0 + 512, :].reshape(PAIRS, 128, DIM)).astype(bf)
        halves.append((wq, wk, wv, wo))
    in_maps = []
    for i in range(NCORES):
        b, hh = i // 2, i % 2
        xt = np.ascontiguousarray(np.asarray(x[b], np.float32).T.astype(bf))
        wq, wk, wv, wo = halves[hh]
        in_maps.append({"xt": xt, "wq": wq, "wk": wk, "wv": wv, "wo": wo})
    return in_maps


def _assemble(results, b_out):
    out = np.empty((B, N, DIM), np.float32)
    bias = np.asarray(b_out, np.float32)
    for b in range(B):
        out[b] = (np.asarray(results[2 * b]["out"], np.float32)
                  + np.asarray(results[2 * b + 1]["out"], np.float32) + bias)
    return out


def run(x, w_qkv, w_out, b_out, trace=False):
    """Run the kernel; returns (output, BassKernelResults)."""
    from concourse.bass_utils import run_bass_kernel_spmd
    nc = _get_nc()
    in_maps = _make_in_maps(x, w_qkv, w_out, b_out)
    res = run_bass_kernel_spmd(nc, in_maps, core_ids=list(range(NCORES)),
                               trace=trace)
    return _assemble(res.results, b_out), res


def kernel(x, w_qkv, w_out, b_out):
    out, _ = run(x, w_qkv, w_out, b_out, trace=False)
    return out
